# revision 38
# baseline (speedup 1.0000x reference)
"""Trainium2 Bass kernel for nn_EquivariantPerturbationTransform.

Reference (N=6000 genes, D=256, H=8 heads, P=128 perturbations, B=16):
  q = H @ Wq.T ; k,v from gathered perturbation rows
  scores[h,n,p] shared across batches; per-batch mask over p (ragged)
  attn_out[b] = softmax-masked attention -> out proj (zeroed for empty b)
  x = LN1(H + attn_out); out = LN2(x + gelu(x@W1.T)@W2.T)

v2 strategy (sequence-parallel over 8 cores, 768 query rows/core):
  - Wo is folded into the values on the host: vo[(h,p),:] = v[p,h] @ Wo_h.T,
    so attention context IS the projected attn_out -- the per-batch [N,D]@[D,D]
    out-projection disappears and the attention matmul lands in row layout.
  - Normalized masked attention weights built per (batch, 16-block):
    a replication matmul (0/1 stationary R_bg) broadcasts reciprocal softmax
    denominators from [(h,b),n] packed layout into [(h,p16),n] block layout
    (zeroing masked rows), then one DVE multiply gives Ehat = E * rep.
  - LN1 in row layout straight from the attention PSUM; x transposed (bf16,
    1 cyc/row) to T layout for FFN1; FFN2 in T layout with the x residual
    accumulated by an identity-stationary matmul; y transposed back for
    LN2 + store.
  - rsqrt for the LNs via a quadratic seed + one Newton step (DVE imm
    tensor_scalars + Pool tensor_tensors) -- keeps Sqrt off the ACT engine so
    the activation table never leaves gelu_and_others after the phase-A Exp.
  - bf16 matmuls everywhere except q/k scores and the H residual (fp32r);
    fp32 PSUM accumulation. H and x residuals enter PSUM as identity-
    stationary matmuls (free on the PE).
  - Emission is software-pipelined per 3-tile group: batch j's attention/LN1
    interleaves with batch j-1's FFN so the in-order PE queue never drains.
    DMAs round-robin over the SP/Pool/ACT trigger queues.
  - Engine split: PE matmuls+transposes; DVE bn_stats, Ehat muls, xT evac;
    ACT exp, gelu, LN applies (per-partition scale/bias), yT evac.
"""

import os
import sys

sys.path.insert(0, "/opt/trn_rl_repo")

import numpy as np

import concourse.bass as bass
from concourse import mybir
from concourse.tile import TileContext

F32 = mybir.dt.float32
FP8 = mybir.dt.float8e4
USE_FP8 = bool(os.environ.get("BASS_FP8"))
F32R = mybir.dt.float32r
BF16 = mybir.dt.bfloat16
AF = mybir.ActivationFunctionType
ALU = mybir.AluOpType

N, D, H, P, B = 6000, 256, 8, 128, 16
DH = D // H  # 32
NCORES = 8
NPAD = 6144
NG = NPAD // NCORES  # 768 rows per core
NT = NG // 128       # 6 row tiles
NCH = 2              # 384-wide chunks
CH = NG // NCH
EPS = 1e-5
GW = 16              # perturbation block width
NGRP = P // GW       # 8 blocks
F1 = 4 * D           # 1024


def _split_waits(nc, max_waits=1):
    """The neuronxcc/walrus build here rejects >1 sync-wait per instruction;
    hoist excess waits onto same-engine NoOps (semantically identical)."""
    n_split = 0
    for f in nc.m.functions:
        for bb in f.blocks:
            new_list = []
            for ins in bb.instructions:
                si = getattr(ins, "sync_info", None)
                if si is not None and si.on_wait and len(si.on_wait) > max_waits:
                    waits = list(si.on_wait)
                    excess, keep = waits[:-max_waits], waits[-max_waits:]
                    for i in range(0, len(excess), max_waits):
                        chunk = excess[i : i + max_waits]
                        nop = mybir.InstNoOp(name=f"{ins.name}-ws{i}", ins=[], outs=[])
                        nop.engine = ins.engine
                        nop.sync_info = mybir.SyncInfo(on_wait=chunk, on_update=[])
                        new_list.append(nop)
                        n_split += 1
                    si.on_wait = keep
                new_list.append(ins)
            bb.instructions = new_list
    return n_split


def _build_program(counts, blocks, nrep, flags):
    """blocks[b] = list of (g, rep_idx) block/replication-matrix pairs."""
    (use_bq, use_b1, use_b2, use_g1, use_b1ln, use_g2, use_b2ln) = flags
    nc = bass.Bass()

    # ---- DRAM parameters -------------------------------------------------
    hgb_row = nc.declare_dram_parameter("hgb_row", [NG, D], F32R, isOutput=False)
    hg_row = nc.declare_dram_parameter("hg_row", [NG, D], F32R, isOutput=False)
    hg_t = nc.declare_dram_parameter("hg_t", [D, NG], F32R, isOutput=False)
    kt = nc.declare_dram_parameter("kt", [D, P], F32R, isOutput=False)
    wq_t = nc.declare_dram_parameter("wq_t", [D, D], F32R, isOutput=False)
    bq_col = nc.declare_dram_parameter("bq_col", [D, 1], F32, isOutput=False)
    vo = nc.declare_dram_parameter("vo", [NGRP, 128, D], BF16, isOutput=False)
    m01bd = nc.declare_dram_parameter("m01bd", [NGRP, 128, 128], BF16, isOutput=False)
    repm = nc.declare_dram_parameter("repm", [max(nrep, 1), 128, 128], BF16, isOutput=False)
    identb = nc.declare_dram_parameter("identb", [128, 128], BF16, isOutput=False)
    identf = nc.declare_dram_parameter("identf", [128, 128], F32, isOutput=False)
    identr = nc.declare_dram_parameter("identr", [128, 128], F32R, isOutput=False)
    emptyp = nc.declare_dram_parameter("emptyp", [128, 1], F32, isOutput=False)
    w1_t = nc.declare_dram_parameter("w1_t", [D, F1], BF16, isOutput=False)
    w1p = nc.declare_dram_parameter("w1p", [128, 2, F1], FP8, isOutput=False)
    w2_t = nc.declare_dram_parameter("w2_t", [F1, D], BF16, isOutput=False)
    b1_col = nc.declare_dram_parameter("b1_col", [F1, 1], F32, isOutput=False)
    ln1_col = nc.declare_dram_parameter("ln1_col", [D, 2], F32, isOutput=False)
    gb_row = nc.declare_dram_parameter("gb_row", [6, D], F32, isOutput=False)
    out = nc.declare_dram_parameter("out", [B, NG, D], F32, isOutput=True)

    s_attn = 1.0 / float(np.sqrt(DH))
    any_empty = any(int(c) == 0 for c in counts)
    jobs = [b for b in range(B) if int(counts[b]) > 0]

    with TileContext(nc) as tc, nc.allow_low_precision(
            reason="bf16 matmul inputs; tolerance budget is 2e-2 of max"):
        import contextlib
        import itertools

        _dmaq = itertools.cycle(
            lambda o, i: nc.sync.dma_start(out=o, in_=i)
            for _ in range(1))
        _dma_engines = None

        def dma(out_ap, in_ap):
            nonlocal _dma_engines
            if _dma_engines is None:
                _dma_engines = itertools.cycle(
                    [nc.sync, nc.gpsimd, nc.scalar])
            next(_dma_engines).dma_start(out=out_ap, in_=in_ap)

        cstack = contextlib.ExitStack()
        consts = cstack.enter_context(tc.tile_pool(name="consts", bufs=1))

        # ---- persistent constants -------------------------------------
        def load_w(name, ap, rows, cols, dt):
            tiles = []
            for k in range(rows // 128):
                tl = consts.tile([128, cols], dt, tag=f"{name}{k}", name=f"{name}{k}")
                dma(tl[:], ap[k * 128 : (k + 1) * 128, :])
                tiles.append(tl)
            return tiles

        hgt_sb = load_w("hgt", hg_t, D, NG, F32R)
        wq_sb = load_w("wq", wq_t, D, D, F32R)
        kt_sb = load_w("kt", kt, D, P, F32R)

        vo_sb = []
        for g in range(NGRP):
            tl = consts.tile([128, D], BF16, tag=f"vo{g}", name=f"vo{g}")
            dma(tl[:], vo[g, :, :])
            vo_sb.append(tl)
        m01_sb = []
        for g in range(NGRP):
            tl = consts.tile([128, 128], BF16, tag=f"m01{g}", name=f"m01{g}")
            dma(tl[:], m01bd[g, :, :])
            m01_sb.append(tl)
        rep_sb = []
        for i in range(nrep):
            tl = consts.tile([128, 128], BF16, tag=f"rep{i}", name=f"rep{i}")
            dma(tl[:], repm[i, :, :])
            rep_sb.append(tl)
        identb_sb = consts.tile([128, 128], BF16, tag="identb", name="identb")
        dma(identb_sb[:], identb[:, :])
        identf_sb = consts.tile([128, 128], F32, tag="identf", name="identf")
        dma(identf_sb[:], identf[:, :])
        identr_sb = consts.tile([128, 128], F32R, tag="identr", name="identr")
        dma(identr_sb[:], identr[:, :])
        empty_sb = consts.tile([128, 1], F32, tag="empty", name="empty")
        dma(empty_sb[:], emptyp[:, :])
        b1_sb = load_w("b1c", b1_col, F1, 1, F32) if use_b1 else None
        bq_sb = load_w("bqc", bq_col, D, 1, F32) if use_bq else None
        ln1_sb = load_w("ln1c", ln1_col, D, 2, F32) if (use_g1 or use_b1ln) else None
        gbr_sb = None
        if use_g2 or use_b2ln or use_b2 or use_g1 or use_b1ln:
            gbr_sb = consts.tile([128, 6, D], F32, tag="gbr", name="gbr")
            nc.gpsimd.dma_start(out=gbr_sb[:], in_=gb_row[:, :].to_broadcast((128, 6, D)))

        # bulk loads (back-half only; issued after the attention-critical set)
        hgb_sb = load_w("hgb", hgb_row, NG, D, F32R)
        if USE_FP8:
            w1p_sb = consts.tile([128, 2, F1], FP8, tag="w1p", name="w1p")
            dma(w1p_sb[:], w1p[:, :, :])
            w1_sb = None
        else:
            w1_sb = load_w("w1", w1_t, D, F1, BF16)
        w2_sb = load_w("w2", w2_t, F1, D, BF16)
        hg_sb = load_w("hg", hg_row, NG, D, F32R) if any_empty else None

        # persistent activations
        qT_sb = [consts.tile([128, NG], F32R, tag=f"qT{i}", name=f"qT{i}")
                 for i in range(2)]
        Et = consts.tile([128, H, NG], BF16, tag="Et", name="Et")
        Eg = [consts.tile([128, NG], BF16, tag=f"Eg{g}", name=f"Eg{g}")
              for g in range(NGRP)]
        den_sb = consts.tile([128, NG], F32, tag="den", name="den")
        denf_sb = consts.tile([128, NG], F32, tag="denf", name="denf")
        denr_sb = consts.tile([128, NG], BF16, tag="denr", name="denr")

        # ================= Phase A: shared projections ==================
        with tc.tile_pool(name="psA", bufs=4, space="PSUM") as psA:
            # qT [D, NG]
            for m in range(2):
                for c in range(NCH):
                    ps = psA.tile([128, CH], F32, tag="ps", name="ps")
                    for kk in range(2):
                        nc.tensor.matmul(
                            ps[:],
                            wq_sb[kk][:, m * 128 : (m + 1) * 128],
                            hgt_sb[kk][:, c * CH : (c + 1) * CH],
                            start=(kk == 0), stop=(kk == 1))
                    if use_bq:
                        nc.scalar.activation(
                            qT_sb[m][:, c * CH : (c + 1) * CH], ps[:], AF.Identity,
                            bias=bq_sb[m][:, 0:1])
                    else:
                        nc.scalar.activation(
                            qT_sb[m][:, c * CH : (c + 1) * CH], ps[:], AF.Copy)

            # E^T per head: Et[p, h, n] = exp(s * k_h q_h^T); each head's
            # block-layout regroup DMAs are issued as soon as it finishes.
            for h in range(H):
                ktt = kt_sb[h // 4]
                for c in range(NCH):
                    ps = psA.tile([128, CH], F32, tag="ps", name="ps")
                    nc.tensor.matmul(
                        ps[:],
                        ktt[(h % 4) * DH : (h % 4 + 1) * DH, :],
                        qT_sb[h // 4][(h % 4) * DH : (h % 4 + 1) * DH,
                                      c * CH : (c + 1) * CH],
                        start=True, stop=True,
                        tile_position=((h % 4) * DH, 0))
                    nc.scalar.activation(Et[:, h, c * CH : (c + 1) * CH],
                                         ps[:], AF.Exp, scale=s_attn)
                for g in range(NGRP):
                    dma(Eg[g][h * GW : (h + 1) * GW, :],
                        Et[g * GW : (g + 1) * GW, h, :])

            # softmax denominators, packed [(h,b), n]; +1 on empty batches
            for c in range(NCH):
                psd = psA.tile([128, CH], F32, tag="psd", name="psd")
                for g in range(NGRP):
                    nc.tensor.matmul(
                        psd[:], m01_sb[g][:], Eg[g][:, c * CH : (c + 1) * CH],
                        start=(g == 0), stop=(g == NGRP - 1))
                nc.vector.tensor_scalar(
                    out=den_sb[:, c * CH : (c + 1) * CH], in0=psd[:],
                    scalar1=empty_sb[:, 0:1], scalar2=None, op0=ALU.add)
                nc.vector.reciprocal(
                    out=denr_sb[:, c * CH : (c + 1) * CH],
                    in_=den_sb[:, c * CH : (c + 1) * CH])

        # ================= Phase B: per-batch back half =================
        # Emission is interleaved at 3-tile-group granularity so the PE
        # always has FFN work of batch j-1 queued behind the attention/LN1
        # work of batch j (in-order engine queues; stalls otherwise).
        work = cstack.enter_context(tc.tile_pool(name="work", bufs=3))
        ehpool = cstack.enter_context(tc.tile_pool(name="ehp", bufs=3))
        ps_ao = cstack.enter_context(tc.tile_pool(name="ps_ao", bufs=3, space="PSUM"))
        ps_tp = cstack.enter_context(tc.tile_pool(name="ps_tp", bufs=3, space="PSUM"))
        ps_ff = cstack.enter_context(tc.tile_pool(name="ps_ff", bufs=2, space="PSUM"))

        RC2, RC1, RC0 = 0.29333931447269, -1.1711876763158582, 1.8939170369253155

        def rsqrt_group(vv_ap, mu_ap, sc_ap, ng_ap, sA, sB):
            """sc = rsqrt(vv), ng = -mu*sc on [128, 3] slices. Quadratic seed
            + one Newton step; tensor_tensor ops run on the idle Pool engine,
            imm tensor_scalar ops on DVE (Pool has no TensorScalar)."""
            nc.vector.tensor_scalar(out=sA, in0=vv_ap, scalar1=RC2,
                                    scalar2=RC1, op0=ALU.mult, op1=ALU.add)
            nc.gpsimd.tensor_tensor(out=sA, in0=sA, in1=vv_ap, op=ALU.mult)
            nc.vector.tensor_scalar(out=sA, in0=sA, scalar1=RC0,
                                    scalar2=None, op0=ALU.add)
            nc.gpsimd.tensor_tensor(out=sB, in0=sA, in1=sA, op=ALU.mult)
            nc.gpsimd.tensor_tensor(out=sB, in0=sB, in1=vv_ap, op=ALU.mult)
            nc.vector.tensor_scalar(out=sB, in0=sB, scalar1=-0.5,
                                    scalar2=1.5, op0=ALU.mult, op1=ALU.add)
            nc.gpsimd.tensor_tensor(out=sc_ap, in0=sA, in1=sB, op=ALU.mult)
            nc.vector.scalar_tensor_tensor(
                out=ng_ap, in0=mu_ap, scalar=-1.0, in1=sc_ap,
                op0=ALU.mult, op1=ALU.mult)

        def front_attn(b, bl):
            """Replicated+masked reciprocal denominators -> Ehat per block."""
            eh = []
            for i, (g, ri) in enumerate(bl):
                tl = ehpool.tile([128, NG], BF16, tag=f"eh{i}", name=f"eh{i}")
                eh.append(tl)
            for c in range(NCH):
                for i, (g, ri) in enumerate(bl):
                    psr = ps_tp.tile([128, CH], F32, tag="tp", name="rep")
                    nc.tensor.matmul(
                        psr[:], rep_sb[ri][:],
                        denr_sb[:, c * CH : (c + 1) * CH],
                        start=True, stop=True)
                    nc.vector.tensor_mul(
                        eh[i][:, c * CH : (c + 1) * CH],
                        Eg[g][:, c * CH : (c + 1) * CH], psr[:])
            return eh

        def front_new(b):
            st = {
                "mvb": work.tile([128, NT, 2], F32, tag="mvb", name="mvb"),
                "sc1": work.tile([128, NT], F32, tag="sc1", name="sc1"),
                "ng1": work.tile([128, NT], F32, tag="ng1", name="ng1"),
                "stats": work.tile([128, 6], F32, tag="stats", name="stats"),
                "x_row": work.tile([128, NT, D], BF16, tag="x_row", name="x_row"),
                "xT": work.tile([128, 2, NG], FP8 if USE_FP8 else BF16,
                                tag="xT", name="xT"),
                "sA": work.tile([128, 3], F32, tag="sA", name="sA"),
                "sB": work.tile([128, 3], F32, tag="sB", name="sB"),
                "pg": {},
            }
            return st

        def front_group(b, bl, st, c):
            """Attention matmuls + H residual + LN1 stats for tiles 3c..3c+2."""
            hrow = hgb_sb if (b is not None) else (hg_sb or hgb_sb)
            for t in range(c * 3, c * 3 + 3):
                psa = ps_ao.tile([128, D], F32, tag="ao", name="ao")
                if bl:
                    for i, (g, ri) in enumerate(bl):
                        nc.tensor.matmul(
                            psa[:], st["eh"][i][:, t * 128 : (t + 1) * 128],
                            vo_sb[g][:], start=(i == 0), stop=False)
                nc.tensor.matmul(psa[:], identr_sb[:], hrow[t][:],
                                 start=(not bl), stop=True)
                nc.vector.bn_stats(out=st["stats"][:], in_=psa[:])
                nc.vector.bn_aggr(out=st["mvb"][:, t, :], in_=st["stats"][:])
                st["pg"][t] = psa

        def front_ln1(b, st, c):
            """rsqrt + LN1 apply for tiles 3c..3c+2 (queued on ACT before the
            previous batch's gelus so the later transposes never wait)."""
            t0 = c * 3
            mvb, sc1, ng1 = st["mvb"], st["sc1"], st["ng1"]
            rsqrt_group(mvb[:, t0 : t0 + 3, 1], mvb[:, t0 : t0 + 3, 0],
                        sc1[:, t0 : t0 + 3], ng1[:, t0 : t0 + 3],
                        st["sA"][:], st["sB"][:])
            for t in range(t0, t0 + 3):
                nc.scalar.activation(
                    st["x_row"][:, t, :], st["pg"].pop(t)[:], AF.Identity,
                    bias=ng1[:, t : t + 1], scale=sc1[:, t : t + 1])

        def front_xp(b, st, c):
            """x transposes + fp8/bf16 evac for tiles 3c..3c+2."""
            for t in range(c * 3, c * 3 + 3):
                psx = ps_tp.tile([128, D], BF16, tag="tp", name="xp")
                for k in range(2):
                    nc.tensor.transpose(
                        psx[:, k * 128 : (k + 1) * 128],
                        st["x_row"][:, t, k * 128 : (k + 1) * 128],
                        identb_sb[:])
                nc.vector.tensor_copy(
                    out=st["xT"][:, :, t * 128 : (t + 1) * 128],
                    in_=psx[:].rearrange("p (k n) -> p k n", k=2))

        def back_ffn1(b, st, half):
            """FFN1 for 4 of the 8 m-chunks, both 384-chunks per stationary
            so each weight load is amortized over 768 moving columns."""
            if half == 0:
                st["h1g"] = work.tile([128, 8, NG], BF16, tag="h1g", name="h1g")
            xT = st["xT"]
            gsc = (1.0 / 16.0) if USE_FP8 else 1.0
            for m in range(half * 4, half * 4 + 4):
                pss = [ps_ff.tile([128, CH], F32, tag="ff", name="f1")
                       for _ in range(2)]
                if USE_FP8:
                    for c in range(NCH):
                        nc.tensor.matmul(
                            pss[c][:], w1p_sb[:, :, m * 128 : (m + 1) * 128],
                            xT[:, :, c * CH : (c + 1) * CH],
                            start=True, stop=True,
                            perf_mode=mybir.MatmulPerfMode.DoubleRow)
                else:
                    for kk in range(2):
                        for c in range(NCH):
                            nc.tensor.matmul(
                                pss[c][:], w1_sb[kk][:, m * 128 : (m + 1) * 128],
                                xT[:, kk, c * CH : (c + 1) * CH],
                                start=(kk == 0), stop=(kk == 1))
                for c in range(NCH):
                    if use_b1:
                        nc.scalar.activation(
                            st["h1g"][:, m, c * CH : (c + 1) * CH],
                            pss[c][:], AF.Gelu, bias=b1_sb[m][:, 0:1],
                            scale=gsc)
                    else:
                        nc.scalar.activation(
                            st["h1g"][:, m, c * CH : (c + 1) * CH],
                            pss[c][:], AF.Gelu, scale=gsc)

        def back_ffn2_ln2(b, st):
            # FFN2 (T layout) + x residual on the PE; ACT evac to bf16
            xres = st["xT"]
            if use_g1 or use_b1ln:
                xres = work.tile([128, 2, NG], BF16, tag="xres", name="xres")
                for k in range(2):
                    nc.vector.tensor_scalar(
                        out=xres[:, k, :], in0=st["xT"][:, k, :],
                        scalar1=ln1_sb[k][:, 0:1], scalar2=ln1_sb[k][:, 1:2],
                        op0=ALU.mult, op1=ALU.add)
            yT = work.tile([128, 2, NG], BF16, tag="yT", name="yT")
            for m in range(2):
                pss = [ps_ff.tile([128, CH], F32, tag="ff", name="y")
                       for _ in range(2)]
                for kk in range(8):
                    for c in range(NCH):
                        nc.tensor.matmul(
                            pss[c][:], w2_sb[kk][:, m * 128 : (m + 1) * 128],
                            st["h1g"][:, kk, c * CH : (c + 1) * CH],
                            start=(kk == 0), stop=False)
                for c in range(NCH):
                    nc.tensor.matmul(
                        pss[c][:], identb_sb[:],
                        xres[:, m, c * CH : (c + 1) * CH],
                        start=False, stop=True)
                    nc.scalar.activation(yT[:, m, c * CH : (c + 1) * CH],
                                         pss[c][:], AF.Copy)

            mv2 = work.tile([128, NT, 2], F32, tag="mv2", name="mv2")
            sc2 = work.tile([128, NT], F32, tag="sc2", name="sc2")
            ng2 = work.tile([128, NT], F32, tag="ng2", name="ng2")
            st2 = work.tile([128, 6], F32, tag="st2", name="st2")
            sA2 = work.tile([128, 3], F32, tag="sA2", name="sA2")
            sB2 = work.tile([128, 3], F32, tag="sB2", name="sB2")
            orow = work.tile([128, NT, D], F32, tag="orow", name="orow")
            for c in range(NCH):
                t0 = c * 3
                pg = []
                for t in range(t0, t0 + 3):
                    psy = ps_tp.tile([128, D], BF16, tag="tp", name="yt")
                    for k in range(2):
                        nc.tensor.transpose(
                            psy[:, k * 128 : (k + 1) * 128],
                            yT[:, k, t * 128 : (t + 1) * 128], identb_sb[:])
                    if use_b2:
                        nc.vector.tensor_add(psy[:], psy[:], gbr_sb[:, 0, :])
                    nc.vector.bn_stats(out=st2[:], in_=psy[:])
                    nc.vector.bn_aggr(out=mv2[:, t, :], in_=st2[:])
                    pg.append(psy)
                rsqrt_group(mv2[:, t0 : t0 + 3, 1], mv2[:, t0 : t0 + 3, 0],
                            sc2[:, t0 : t0 + 3], ng2[:, t0 : t0 + 3],
                            sA2[:], sB2[:])
                for t in range(t0, t0 + 3):
                    nc.scalar.activation(
                        orow[:, t, :], pg[t - t0][:], AF.Identity,
                        bias=ng2[:, t : t + 1], scale=sc2[:, t : t + 1])
                    if use_g2:
                        nc.vector.tensor_mul(orow[:, t, :], orow[:, t, :],
                                             gbr_sb[:, 2, :])
                    if use_b2ln:
                        nc.vector.tensor_add(orow[:, t, :], orow[:, t, :],
                                             gbr_sb[:, 3, :])
            outv = out[b if b is not None else 0].rearrange(
                "(t p) d -> p t d", p=128)
            if b is not None:
                dma(outv, orow[:])
            else:
                for be in range(B):
                    if int(counts[be]) == 0:
                        dma(out[be].rearrange("(t p) d -> p t d", p=128),
                            orow[:])

        alljobs = [(b, blocks[b]) for b in jobs]
        if any_empty:
            alljobs.append((None, []))
        prev = None
        for b, bl in alljobs:
            st = front_new(b)
            st["eh"] = front_attn(b, bl) if bl else []
            front_group(b, bl, st, 0)
            front_ln1(b, st, 0)
            if prev is not None:
                back_ffn1(prev[0], prev[1], 0)
            front_group(b, bl, st, 1)
            front_xp(b, st, 0)
            front_ln1(b, st, 1)
            if prev is not None:
                back_ffn1(prev[0], prev[1], 1)
            front_xp(b, st, 1)
            if prev is not None:
                back_ffn2_ln2(prev[0], prev[1])
            prev = (b, st)
        back_ffn1(prev[0], prev[1], 0)
        back_ffn1(prev[0], prev[1], 1)
        back_ffn2_ln2(prev[0], prev[1])

        cstack.close()

    return nc


def kernel(H_genes, perturbation_indices, batch_assignment, batch_size,
           in_proj_w, in_proj_b, out_proj_w, out_proj_b,
           ffn_w1, ffn_b1, ffn_w2, ffn_b2,
           ln1_g, ln1_b, ln2_g, ln2_b):
    import ml_dtypes
    bf16 = ml_dtypes.bfloat16

    Hg = np.ascontiguousarray(np.asarray(H_genes, dtype=np.float32))
    pidx = np.asarray(perturbation_indices).astype(np.int64)
    ba = np.asarray(batch_assignment).astype(np.int64)
    Bs = int(np.asarray(batch_size))
    assert Bs == B, f"kernel hardcodes B=16, got {Bs}"
    assert Hg.shape == (N, D)

    Wq, Wk, Wv = [np.asarray(w, np.float32) for w in np.split(np.asarray(in_proj_w), 3, axis=0)]
    bq, bk, bv = [np.asarray(x, np.float32) for x in np.split(np.asarray(in_proj_b), 3, axis=0)]
    Wo = np.asarray(out_proj_w, np.float32)
    bo = np.asarray(out_proj_b, np.float32)
    W1 = np.asarray(ffn_w1, np.float32)
    b1 = np.asarray(ffn_b1, np.float32)
    W2 = np.asarray(ffn_w2, np.float32)
    b2 = np.asarray(ffn_b2, np.float32)
    g1 = np.asarray(ln1_g, np.float32)
    be1 = np.asarray(ln1_b, np.float32)
    g2 = np.asarray(ln2_g, np.float32)
    be2 = np.asarray(ln2_b, np.float32)

    counts = np.bincount(ba, minlength=B).astype(np.int64)
    has_any = counts > 0

    # host-side small projections: k and Wo-folded values
    Hp = Hg[pidx]                                   # [P, D]
    k = Hp @ Wk.T + bk[None, :]                     # [P, D]
    v = Hp @ Wv.T + bv[None, :]                     # [P, D]
    # vo[(h,p),:] = v[p, h-slice] @ Wo[:, h-slice].T  (full attn_out proj)
    vo = np.zeros((NGRP, 128, D), np.float32)
    for g in range(NGRP):
        for h in range(H):
            vh = v[g * GW : (g + 1) * GW, h * DH : (h + 1) * DH]   # [16, 32]
            vo[g, h * GW : (h + 1) * GW, :] = vh @ Wo[:, h * DH : (h + 1) * DH].T

    # block-diagonal per-head mask matrices for denominators
    m01 = (ba[:, None] == np.arange(B)[None, :]).astype(np.float32)  # [P, B]
    m01bd = np.zeros((NGRP, 128, 128), np.float32)
    for g in range(NGRP):
        for h in range(H):
            m01bd[g, h * GW : (h + 1) * GW, h * GW : (h + 1) * GW] = \
                m01[g * GW : (g + 1) * GW, :]

    # replication matrices: rep[(h,b'),(h,p16)] = 1{b'==b and ba[p]==b}
    blocks = {b: [] for b in range(B)}
    rep_mats = []
    for b in range(B):
        if counts[b] == 0:
            continue
        for g in range(NGRP):
            psel = np.where(ba[g * GW : (g + 1) * GW] == b)[0]
            if len(psel) == 0:
                continue
            R = np.zeros((128, 128), np.float32)
            for h in range(H):
                for p16 in psel:
                    R[h * GW + b, h * GW + p16] = 1.0
            blocks[b].append((g, len(rep_mats)))
            rep_mats.append(R)
    nrep = len(rep_mats)

    # fold ln1 gain into FFN1 (exact): W1' = W1*g1, b1' = W1@b1_ln + b1
    W1f = W1 * g1[None, :]
    b1f = b1 + W1 @ be1

    Hg_pad = np.zeros((NPAD, D), np.float32)
    Hg_pad[:N] = Hg
    emptyp = np.tile((~has_any).astype(np.float32), H)[:, None]  # [(h,b),1]

    flags = (
        bool(np.any(bq != 0)), bool(np.any(b1f != 0)), bool(np.any(b2 != 0)),
        bool(np.any(g1 != 1)), bool(np.any(be1 != 0)),
        bool(np.any(g2 != 1)), bool(np.any(be2 != 0)),
    )
    use_bo = bool(np.any(bo != 0))

    nc = _build_program(counts, blocks, nrep, flags)

    common = {
        "kt": np.ascontiguousarray(k.T),
        "wq_t": np.ascontiguousarray(Wq.T),
        "bq_col": bq[:, None].copy(),
        "vo": vo.astype(bf16),
        "m01bd": m01bd.astype(bf16),
        "repm": (np.stack(rep_mats) if nrep else np.zeros((1, 128, 128), np.float32)).astype(bf16),
        "identb": np.eye(128, dtype=np.float32).astype(bf16),
        "identf": np.eye(128, dtype=np.float32),
        "identr": np.eye(128, dtype=np.float32),
        "emptyp": np.ascontiguousarray(emptyp),
        "w1_t": np.ascontiguousarray(W1f.T).astype(bf16),
        "w1p": np.ascontiguousarray(
            (W1f.T * 16.0).reshape(2, 128, F1).transpose(1, 0, 2)
        ).astype(ml_dtypes.float8_e4m3fn),
        "w2_t": np.ascontiguousarray(W2.T).astype(bf16),
        "b1_col": b1f[:, None].copy(),
        "ln1_col": np.ascontiguousarray(np.stack([g1, be1], axis=1)),
        "gb_row": np.stack([b2, be1, g2, be2, g1, be1], axis=0),
    }
    in_maps = []
    for c in range(NCORES):
        sl = Hg_pad[c * NG : (c + 1) * NG]
        m = dict(common)
        m["hg_row"] = np.ascontiguousarray(sl)
        m["hgb_row"] = np.ascontiguousarray(sl + bo[None, :]) if use_bo else m["hg_row"]
        m["hg_t"] = np.ascontiguousarray(sl.T)
        in_maps.append(m)

    if os.environ.get("BASS_KERNEL_SIM"):
        from concourse import bass_interp
        # CoreSim lacks a Gelu LUT; shim exact (erf) gelu for local debugging.
        if not getattr(bass_interp.InstructionExecutor, "_gelu_patched", False):
            from scipy.special import erf
            _orig_act = bass_interp.InstructionExecutor.visit_InstActivation

            def _act(self, instruction, *, reg_snapshot=None):
                if instruction.func == mybir.ActivationFunctionType.Gelu:
                    instruction.func = mybir.ActivationFunctionType.Identity
                    try:
                        import concourse.bass_interp as bi
                        out_ap = instruction.outs[0]
                        r = _orig_act(self, instruction, reg_snapshot=reg_snapshot)
                        view = self.view_ap(out_ap, bi.Direction.READ, instruction,
                                            reg_snapshot=reg_snapshot)
                        x = view.astype(np.float64)
                        view[:] = (0.5 * x * (1.0 + erf(x / np.sqrt(2.0)))).astype(view.dtype)
                        return r
                    finally:
                        instruction.func = mybir.ActivationFunctionType.Gelu
                return _orig_act(self, instruction, reg_snapshot=reg_snapshot)

            bass_interp.InstructionExecutor.visit_InstActivation = _act
            bass_interp.InstructionExecutor._gelu_patched = True
        nsim = int(os.environ.get("BASS_KERNEL_SIM_CORES", "1"))
        simtrace = bool(os.environ.get("BASS_KERNEL_SIMTRACE"))
        sim = bass_interp.MultiCoreSim(nc, nsim, trace=simtrace)
        for c in range(nsim):
            for kk, vv in in_maps[c].items():
                sim.cores[c].tensor(kk)[:] = vv
        sim.simulate()
        print(f"SIM predicted time: {sim.cores[0].time} ns")
        full = np.zeros((B, NPAD, D), np.float32)
        for c in range(nsim):
            full[:, c * NG : (c + 1) * NG, :] = (
                np.array(sim.cores[c].mem_tensor("out")).reshape(B, NG, D))
        return full[:, :N, :]

    from concourse.bass_utils import run_bass_kernel_spmd
    _split_waits(nc)
    trace = bool(os.environ.get("BASS_KERNEL_TRACE"))
    res = run_bass_kernel_spmd(nc, in_maps, core_ids=list(range(NCORES)),
                               trace=trace)
    if trace and res.exec_time_ns is not None:
        print(f"HW exec time: {res.exec_time_ns} ns")
        if res.instructions_and_trace:
            print("trace:", res.instructions_and_trace[1])

    full = np.zeros((B, NPAD, D), np.float32)
    for c in range(NCORES):
        full[:, c * NG : (c + 1) * NG, :] = res.results[c]["out"]
    return full[:, :N, :]
